# revision 4
# baseline (speedup 1.0000x reference)
"""Trainium2 Bass kernel for nn_DiscriptorMatchLoss (retrieval_knn).

loss = mean over matched pairs of (1 - cos(desc_src, desc_dst)), where a
match is dist(ps[b,n], pd[a,b,m]) <= 1 pixel AND n < m (strict upper tri).

Strategy (per sharding hint): shard the pair axis `a` across 8 cores; each
core handles the 8 pairs (a, b=0..7). Descriptors (normalized, bf16) are
replicated. Per core:
  - dist2[n, m] via K=4 fp32 PE matmul: [x, y, |p|^2, 1] . [-2x', -2y', 1, |p'|^2]
    (only the strip m >= 128*i is computed for src tile i: lower-tri skipped)
  - mask M = (dist2 <= thr) on DVE (thr encodes radius + strict-upper-tri for
    the diagonal block), bf16 0/1, with per-partition match counts via the
    fused accum_out (free).
  - T[d, m] = sum_n M[n, m] * dhat_b[n, d] via bf16 PE matmuls accumulated in
    PSUM (applies the mask without materializing the cosine matrix).
  - masked cos sum = sum(T * dhat_a^T) via fused tensor_tensor_reduce.
  - partition-reduce via a tiny ones-matmul; DMA [cos_sum, count] out.
Host: loss = (sum(count) - sum(cos_sum)) / sum(count).
"""
import numpy as np
import orjson
import ml_dtypes

import concourse.bass as bass
import concourse.tile as tile
from concourse import mybir
from concourse.bass_utils import run_bass_kernel_spmd

B, N, D = 8, 1024, 256
NT = N // 128  # src tiles per frame
NEG = -1.0e30


# ---------------------------------------------------------------------------
# This container's walrus encodes at most 1 sync-wait per instruction (2 for
# EventSemaphore); Tile can attach more (tail drain, fp32 merged LDW+MM).
# Hoist excess waits onto standalone EventSemaphore instructions right before
# the offending instruction on the same engine (identical blocking semantics).
def _split_waits(bir: dict) -> None:
    uid = [0]

    def mk(engine, debug, waits):
        uid[0] += 1
        return {
            "debug": debug,
            "engine": engine,
            "ins": [],
            "name": f"W-fix-{uid[0]}",
            "opcode": "EventSemaphore",
            "outs": [],
            "sync_info": {"on_update": [], "on_wait": waits},
        }

    for fn in bir.get("functions", []):
        for blk in fn.get("blocks", []):
            out = []
            for ins in blk.get("instructions", []):
                si = ins.get("sync_info")
                waits = (si or {}).get("on_wait") or []
                cap = 2 if ins.get("opcode") == "EventSemaphore" else 1
                if len(waits) > cap:
                    extra = waits[cap:]
                    si["on_wait"] = waits[:cap]
                    for j in range(0, len(extra), 2):
                        out.append(mk(ins.get("engine"), ins.get("debug", 0), extra[j : j + 2]))
                out.append(ins)
            blk["instructions"] = out


class FixedBass(bass.Bass):
    def to_json_bytes(self) -> bytes:
        bir = orjson.loads(super().to_json_bytes())
        _split_waits(bir)
        return orjson.dumps(bir)


def _chunks512(w):
    # split [0, w) into PSUM-bank-aligned pieces of <= 512
    out = []
    off = 0
    while off < w:
        ln = min(512, w - off)
        out.append((off, ln))
        off += ln
    return out


def _build():
    f32, bf16 = mybir.dt.float32, mybir.dt.bfloat16
    nc = FixedBass(trn_type="TRN2")
    sfeat = nc.dram_tensor("sfeat", [4, B, N], f32, kind="ExternalInput")
    rfeat = nc.dram_tensor("rfeat", [4, B, N], f32, kind="ExternalInput")
    thr = nc.dram_tensor("thr", [128, N], f32, kind="ExternalInput")
    dh = nc.dram_tensor("dh", [128, B, NT, D], bf16, kind="ExternalInput")
    dhT = nc.dram_tensor("dhT", [128, 2, N], bf16, kind="ExternalInput")
    out = nc.dram_tensor("out", [2, 1], f32, kind="ExternalOutput")

    with tile.TileContext(nc) as tc:
        with (
            tc.tile_pool(name="const", bufs=1) as cpool,
            tc.tile_pool(name="dhp", bufs=1) as dhpool,
            tc.tile_pool(name="mask", bufs=3) as mpool,
            tc.tile_pool(name="tt", bufs=2) as ttpool,
            tc.tile_pool(name="fin", bufs=1) as fin,
            tc.tile_pool(name="pdist", bufs=2, space="PSUM") as pdp,
            tc.tile_pool(name="pT", bufs=1, space="PSUM") as pTp,
        ):
            sf = cpool.tile([4, B * N], f32)
            nc.sync.dma_start(sf[:], sfeat.rearrange("p b n -> p (b n)")[:])
            rf = cpool.tile([4, B * N], f32)
            nc.sync.dma_start(rf[:], rfeat.rearrange("p b n -> p (b n)")[:])
            th = cpool.tile([128, N], f32)
            nc.sync.dma_start(th[:], thr[:])
            dT = cpool.tile([128, 2 * N], bf16)
            nc.sync.dma_start(dT[:], dhT.rearrange("p c n -> p (c n)")[:])
            dhb = []
            for b in range(B):
                t = dhpool.tile([128, NT * D], bf16, name=f"dh{b}")
                nc.sync.dma_start(t[:], dh[:, b, :, :].rearrange("p i d -> p (i d)"))
                dhb.append(t)

            count_acc = fin.tile([128, B * NT], f32)
            cos_acc = fin.tile([128, B * 2], f32)

            for pb in range(B):
                b = pb
                Tps = pTp.tile([128, 2 * N], f32)  # [d-chunk c, m] accum
                for i in range(NT):
                    m0 = 128 * i
                    w = N - m0
                    pd = pdp.tile([128, N], f32)
                    for off, ln in _chunks512(w):
                        nc.tensor.matmul(
                            pd[:, off : off + ln],
                            sf[:, b * N + 128 * i : b * N + 128 * (i + 1)],
                            rf[:, b * N + m0 + off : b * N + m0 + off + ln],
                            start=True,
                            stop=True,
                        )
                    mt = mpool.tile([128, N], bf16)
                    nc.vector.scalar_tensor_tensor(
                        out=mt[:, 0:w],
                        in0=pd[:, 0:w],
                        scalar=1.0,
                        in1=th[:, 0:w],
                        op0=mybir.AluOpType.mult,
                        op1=mybir.AluOpType.is_le,
                        accum_out=count_acc[:, pb * NT + i : pb * NT + i + 1],
                    )
                    for c in range(2):
                        for off, ln in _chunks512(w):
                            # absolute m-range of this chunk
                            a0 = m0 + off
                            is_first = i == 0
                            # last writer of columns [a0, a0+ln): src tile j
                            # touches m >= 128*j, so last toucher of column m
                            # is j = min(m // 128, NT - 1)
                            last_i = min((a0 + ln - 1) // 128, NT - 1)
                            nc.tensor.matmul(
                                Tps[:, c * N + a0 : c * N + a0 + ln],
                                dhb[b][:, i * D + c * 128 : i * D + (c + 1) * 128],
                                mt[:, off : off + ln],
                                start=is_first,
                                stop=(i == last_i),
                            )
                for c in range(2):
                    tt = ttpool.tile([128, N], bf16)
                    nc.vector.scalar_tensor_tensor(
                        out=tt[:],
                        in0=Tps[:, c * N : (c + 1) * N],
                        scalar=1.0,
                        in1=dT[:, c * N : (c + 1) * N],
                        op0=mybir.AluOpType.mult,
                        op1=mybir.AluOpType.mult,
                        accum_out=cos_acc[:, pb * 2 + c : pb * 2 + c + 1],
                    )

            red = fin.tile([128, 2], f32)
            nc.vector.reduce_sum(red[:, 0:1], cos_acc[:], axis=mybir.AxisListType.X)
            nc.vector.reduce_sum(red[:, 1:2], count_acc[:], axis=mybir.AxisListType.X)
            ones = fin.tile([128, 1], f32)
            nc.vector.memset(ones[:], 1.0)
            ops = pdp.tile([2, 1], f32, tag="pd")
            nc.tensor.matmul(ops[:], red[:], ones[:], start=True, stop=True)
            osb = fin.tile([2, 1], f32)
            nc.vector.tensor_copy(osb[:], ops[:])
            nc.sync.dma_start(out[:], osb[:])
    return nc


_CACHE = {}


def _get_nc():
    if "nc" not in _CACHE:
        _CACHE["nc"] = _build()
    return _CACHE["nc"]


def kernel(descriptors, pts_src, pts_dst, invis_idx, height, width, **_unused):
    del invis_idx
    h = int(np.asarray(height))
    w = int(np.asarray(width))
    descriptors = np.asarray(descriptors, np.float32)
    pts_src = np.asarray(pts_src, np.float32)
    pts_dst = np.asarray(pts_dst, np.float32)

    scale = np.array([(w - 1) * 0.5, (h - 1) * 0.5], np.float32)
    ps = (pts_src + np.float32(1.0)) * scale  # [B, N, 2]
    pdst = (pts_dst + np.float32(1.0)) * scale  # [B, B, N, 2]
    sq_s = (ps * ps).sum(-1)  # [B, N] fp32
    sq_d = (pdst * pdst).sum(-1)  # [B, B, N]

    sfeat = np.stack(
        [ps[:, :, 0], ps[:, :, 1], sq_s, np.ones((B, N), np.float32)], axis=0
    ).astype(np.float32)  # [4, B, N]
    # rfeat per core a: rows [-2x', -2y', 1, |p'|^2] for pd[a, b, m]
    rfeat_all = np.stack(
        [
            np.float32(-2.0) * pdst[..., 0],
            np.float32(-2.0) * pdst[..., 1],
            np.ones((B, B, N), np.float32),
            sq_d,
        ],
        axis=0,
    ).astype(np.float32)  # [4, A, B, N]

    # normalized descriptors
    d64 = descriptors.astype(np.float64)
    nrm = np.sqrt((d64 * d64).sum(-1, keepdims=True))
    dhat = (d64 / nrm).astype(ml_dtypes.bfloat16)  # [B, N, D]
    # dh[p, b, i, d] = dhat[b, 128 i + p, d]
    dh = np.ascontiguousarray(
        dhat.reshape(B, NT, 128, D).transpose(2, 0, 1, 3)
    )  # [128, B, NT, D]
    # dhT[a][p, c, m] = dhat[a, m, 128 c + p]
    dhT_all = np.ascontiguousarray(
        dhat.transpose(0, 2, 1).reshape(B, 2, 128, N).transpose(0, 2, 1, 3)
    )  # [A, 128, 2, N]

    thr = np.full((128, N), 1.0, np.float32)
    diag = np.where(
        np.arange(128)[:, None] < np.arange(128)[None, :], np.float32(1.0), np.float32(NEG)
    )
    thr[:, 0:128] = diag

    nc = _get_nc()
    in_maps = []
    for a in range(8):
        in_maps.append(
            {
                "sfeat": sfeat,
                "rfeat": np.ascontiguousarray(rfeat_all[:, a]),
                "thr": thr,
                "dh": dh,
                "dhT": dhT_all[a],
            }
        )
    _CACHE["last_in_maps"] = in_maps
    res = run_bass_kernel_spmd(nc, in_maps, core_ids=list(range(8)))
    cos_sum = 0.0
    count = 0.0
    for r in res.results:
        cos_sum += float(r["out"][0, 0])
        count += float(r["out"][1, 0])
    return np.float32((count - cos_sum) / count)


# revision 5
# speedup vs baseline: 1.7697x; 1.7697x over previous
"""Trainium2 Bass kernel for nn_DiscriptorMatchLoss (retrieval_knn).

loss = mean over matched pairs of (1 - cos(desc_src, desc_dst)), where a
match is dist(ps[b,n], pd[a,b,m]) <= 1 pixel AND n < m (strict upper tri).

Sharding (per hint): pair axis `a` across 8 cores; core a handles pairs
(a, b=0..7); normalized descriptors replicated (bf16). Per core:
  - dist2'[n, m] (1/64-pixel^2 units) via a K=22 fp16 PE matmul: coordinates
    are split hi/mid/lo (exact fp16 chunks) so products are exact and the
    row order makes partial sums cancel early -> near-threshold error ~2e-5
    while running at 1 cycle/column (4x faster than fp32 matmul).
    Only the strip m >= 128*i is computed for src tile i (lower-tri skipped).
  - mask M = (dist2' <= thr) on DVE; thr encodes radius^2/64 + strict-upper
    tri for the diagonal block; bf16 0/1 output, fused per-partition match
    counts via accum_out (free).
  - T[d, m] = sum_n M[n, m] * dhat_b[n, d] via bf16 PE matmuls accumulated
    in PSUM (applies the mask without materializing the cosine matrix).
  - ScalarE copies T PSUM->SBUF (bf16); DVE computes sum(T * dhat_a^T) via
    fused scalar_tensor_tensor accum.
  - partition-reduce via a tiny ones-matmul; DMA [cos_sum, count] out.
Host: loss = (sum(count) - sum(cos_sum)) / sum(count).
"""
import numpy as np
import orjson
import ml_dtypes

import concourse.bass as bass
import concourse.tile as tile
from concourse import mybir
from concourse.bass_utils import run_bass_kernel_spmd

B, N, D = 8, 1024, 256
NT = N // 128
K22 = 22
NEG = -1.0e30
THR = 1.0 / 64.0  # (radius/8)^2


# ---------------------------------------------------------------------------
# This container's walrus encodes at most 1 sync-wait per instruction (2 for
# EventSemaphore); Tile can attach more (tail drain, merged LDW+MM). Hoist
# excess waits onto standalone EventSemaphore instructions right before the
# offending instruction on the same engine (identical blocking semantics).
def _split_waits(bir: dict) -> None:
    uid = [0]

    def mk(engine, debug, waits):
        uid[0] += 1
        return {
            "debug": debug,
            "engine": engine,
            "ins": [],
            "name": f"W-fix-{uid[0]}",
            "opcode": "EventSemaphore",
            "outs": [],
            "sync_info": {"on_update": [], "on_wait": waits},
        }

    for fn in bir.get("functions", []):
        for blk in fn.get("blocks", []):
            out = []
            for ins in blk.get("instructions", []):
                si = ins.get("sync_info")
                waits = (si or {}).get("on_wait") or []
                cap = 2 if ins.get("opcode") == "EventSemaphore" else 1
                if len(waits) > cap:
                    extra = waits[cap:]
                    si["on_wait"] = waits[:cap]
                    for j in range(0, len(extra), 2):
                        out.append(mk(ins.get("engine"), ins.get("debug", 0), extra[j : j + 2]))
                out.append(ins)
            blk["instructions"] = out


class FixedBass(bass.Bass):
    def to_json_bytes(self) -> bytes:
        bir = orjson.loads(super().to_json_bytes())
        _split_waits(bir)
        return orjson.dumps(bir)


def _chunks512(w):
    out = []
    off = 0
    while off < w:
        ln = min(512, w - off)
        out.append((off, ln))
        off += ln
    return out


def _build():
    f32, bf16, fp16 = mybir.dt.float32, mybir.dt.bfloat16, mybir.dt.float16
    nc = FixedBass(trn_type="TRN2")
    sfeat = nc.dram_tensor("sfeat", [K22, B, N], fp16, kind="ExternalInput")
    rfeat = nc.dram_tensor("rfeat", [K22, B, N], fp16, kind="ExternalInput")
    thr = nc.dram_tensor("thr", [128, N], f32, kind="ExternalInput")
    dh = nc.dram_tensor("dh", [128, B, NT, D], bf16, kind="ExternalInput")
    dhT = nc.dram_tensor("dhT", [128, 2, N], bf16, kind="ExternalInput")
    out = nc.dram_tensor("out", [2, 1], f32, kind="ExternalOutput")

    with tile.TileContext(nc) as tc:
        with (
            tc.tile_pool(name="const", bufs=1) as cpool,
            tc.tile_pool(name="dhp", bufs=1) as dhpool,
            tc.tile_pool(name="mask", bufs=4) as mpool,
            tc.tile_pool(name="tt", bufs=2) as ttpool,
            tc.tile_pool(name="tsb", bufs=2) as tsbpool,
            tc.tile_pool(name="fin", bufs=1) as fin,
            tc.tile_pool(name="pdist", bufs=2, space="PSUM") as pdp,
            tc.tile_pool(name="pT", bufs=1, space="PSUM") as pTp,
        ):
            sf = cpool.tile([K22, B, N], fp16)
            nc.sync.dma_start(sf[:], sfeat[:])
            rf = cpool.tile([K22, B, N], fp16)
            nc.sync.dma_start(rf[:], rfeat[:])
            th = cpool.tile([128, N], f32)
            nc.sync.dma_start(th[:], thr[:])
            dT = cpool.tile([128, 2, N], bf16)
            nc.sync.dma_start(dT[:], dhT[:])
            dhb = []
            for b in range(B):
                t = dhpool.tile([128, NT, D], bf16, name=f"dh{b}")
                nc.sync.dma_start(t[:], dh[:, b, :, :])
                dhb.append(t)

            count_acc = fin.tile([128, B * NT], f32)
            cos_acc = fin.tile([128, B * 2], f32)

            for pb in range(B):
                b = pb
                Tps = pTp.tile([128, 2, N], f32)
                for i in range(NT):
                    m0 = 128 * i
                    w = N - m0
                    pd = pdp.tile([128, N], f32)
                    for off, ln in _chunks512(w):
                        nc.tensor.matmul(
                            pd[:, off : off + ln],
                            sf[:, b, 128 * i : 128 * (i + 1)],
                            rf[:, b, m0 + off : m0 + off + ln],
                            start=True,
                            stop=True,
                        )
                    mt = mpool.tile([128, N], bf16)
                    nc.vector.scalar_tensor_tensor(
                        out=mt[:, 0:w],
                        in0=pd[:, 0:w],
                        scalar=1.0,
                        in1=th[:, 0:w],
                        op0=mybir.AluOpType.mult,
                        op1=mybir.AluOpType.is_le,
                        accum_out=count_acc[:, pb * NT + i : pb * NT + i + 1],
                    )
                    for c in range(2):
                        for off, ln in _chunks512(w):
                            a0 = m0 + off
                            last_i = min((a0 + ln - 1) // 128, NT - 1)
                            nc.tensor.matmul(
                                Tps[:, c, a0 : a0 + ln],
                                dhb[b][:, i, c * 128 : (c + 1) * 128],
                                mt[:, off : off + ln],
                                start=(i == 0),
                                stop=(i == last_i),
                            )
                for c in range(2):
                    tsb = tsbpool.tile([128, N], bf16)
                    nc.scalar.copy(tsb[:], Tps[:, c, :])
                    tt = ttpool.tile([128, N], bf16)
                    nc.vector.scalar_tensor_tensor(
                        out=tt[:],
                        in0=tsb[:],
                        scalar=1.0,
                        in1=dT[:, c, :],
                        op0=mybir.AluOpType.mult,
                        op1=mybir.AluOpType.mult,
                        accum_out=cos_acc[:, pb * 2 + c : pb * 2 + c + 1],
                    )

            red = fin.tile([128, 2], f32)
            nc.vector.reduce_sum(red[:, 0:1], cos_acc[:], axis=mybir.AxisListType.X)
            nc.vector.reduce_sum(red[:, 1:2], count_acc[:], axis=mybir.AxisListType.X)
            ones = fin.tile([128, 1], f32)
            nc.vector.memset(ones[:], 1.0)
            ops = pdp.tile([2, 1], f32, tag="pd")
            nc.tensor.matmul(ops[:], red[:], ones[:], start=True, stop=True)
            osb = fin.tile([2, 1], f32)
            nc.vector.tensor_copy(osb[:], ops[:])
            nc.sync.dma_start(out[:], osb[:])
    return nc


_CACHE = {}


def _get_nc():
    if "nc" not in _CACHE:
        _CACHE["nc"] = _build()
    return _CACHE["nc"]


def _split3(v):
    # v (f64) -> exact-int fp16 + fp16 mid + fp16 lo chunks
    a = np.rint(v)
    b = (v - a).astype(np.float16)
    c = (v - a - b.astype(np.float64)).astype(np.float16)
    return a.astype(np.float16), b, c


def _splitsq(v):
    v1 = np.rint(v / 8.0) * 8.0
    v2 = (v - v1).astype(np.float16)
    v3 = (v - v1 - v2.astype(np.float64)).astype(np.float16)
    return v1.astype(np.float16), v2, v3


def _feat22(u):
    """u: [..., 2] float64 coords (1/8-pixel). Returns (F, R) each [22, ...]."""
    ax, bx, cx = _split3(u[..., 0])
    ay, by, cy = _split3(u[..., 1])
    s1, s2, s3 = _splitsq((u * u).sum(-1))
    one = np.ones_like(ax)
    m2 = np.float16(-2.0)
    Frows = [s1, ax, one, ay, s2, bx, ax, one, by, ay, s3, one,
             bx, by, ax, cx, ay, cy, bx, cx, by, cy]
    Rrows = [one, m2 * ax, s1, m2 * ay, one, m2 * ax, m2 * bx, s2,
             m2 * ay, m2 * by, one, s3, m2 * bx, m2 * by,
             m2 * cx, m2 * ax, m2 * cy, m2 * ay, m2 * cx, m2 * bx, m2 * cy, m2 * by]
    F = np.stack(Frows).astype(np.float16)
    R = np.stack(Rrows).astype(np.float16)
    return F, R


def kernel(descriptors, pts_src, pts_dst, invis_idx, height, width, **_unused):
    del invis_idx
    h = int(np.asarray(height))
    w = int(np.asarray(width))
    descriptors = np.asarray(descriptors, np.float32)
    pts_src = np.asarray(pts_src, np.float32)
    pts_dst = np.asarray(pts_dst, np.float32)

    scale = np.array([(w - 1) * 0.5, (h - 1) * 0.5], np.float32)
    ps = (pts_src + np.float32(1.0)) * scale  # [B, N, 2] fp32 (matches reference)
    pdst = (pts_dst + np.float32(1.0)) * scale  # [A, B, N, 2]

    us = ps.astype(np.float64) * 0.125
    ud = pdst.astype(np.float64) * 0.125
    Fs, _ = _feat22(us)  # [22, B, N] (src uses F rows)
    _, Rd = _feat22(ud)  # [22, A, B, N] (dst uses R rows)
    sfeat = np.ascontiguousarray(Fs)
    rfeat_all = np.ascontiguousarray(Rd)

    d64 = descriptors.astype(np.float64)
    nrm = np.sqrt((d64 * d64).sum(-1, keepdims=True))
    dhat = (d64 / nrm).astype(ml_dtypes.bfloat16)  # [B, N, D]
    dh = np.ascontiguousarray(dhat.reshape(B, NT, 128, D).transpose(2, 0, 1, 3))
    dhT_all = np.ascontiguousarray(
        dhat.transpose(0, 2, 1).reshape(B, 2, 128, N).transpose(0, 2, 1, 3)
    )

    thr = np.full((128, N), THR, np.float32)
    thr[:, 0:128] = np.where(
        np.arange(128)[:, None] < np.arange(128)[None, :], np.float32(THR), np.float32(NEG)
    )

    nc = _get_nc()
    in_maps = []
    for a in range(8):
        in_maps.append(
            {
                "sfeat": sfeat,
                "rfeat": np.ascontiguousarray(rfeat_all[:, a]),
                "thr": thr,
                "dh": dh,
                "dhT": dhT_all[a],
            }
        )
    _CACHE["last_in_maps"] = in_maps
    res = run_bass_kernel_spmd(nc, in_maps, core_ids=list(range(8)))
    cos_sum = 0.0
    count = 0.0
    for r in res.results:
        cos_sum += float(r["out"][0, 0])
        count += float(r["out"][1, 0])
    return np.float32((count - cos_sum) / count)
